# revision 9
# baseline (speedup 1.0000x reference)
"""Trainium2 Bass kernel for a device-aware top-1 MoE layer.

Strategy (expert parallelism over 8 NeuronCores):
  - Host: compute gate logits + top-1 routing (this is the "dispatch"
    step of the sharding), gather each expert's tokens, pad to a common
    capacity C, and transpose to feature-major [D, C] so the device
    matmuls need no on-chip transposes.
  - Device (SPMD, one NEFF on 8 cores): core i holds experts (2i, 2i+1)
    in bf16. For each expert:  hT = relu(w1.T-chunks @ xT + b1),
    yT = w2.T-chunks @ hT + b2, with fp32 PSUM accumulation.
    Activations stay [feature, token] so biases are per-partition.
  - Host: scatter each expert's [D, count] output back to token rows.

Perf notes:
  - Weights are bf16 (fp32 matmul is quarter-rate on the PE and doubles
    HBM traffic; fp32 PSUM accumulation keeps rel err ~3e-3).
  - The kernel is HBM-bound (~18 MB/core at ~360 GB/s/core). All weight
    DMA rides the sync HWDGE queue in exact consumption order; stage 2
    iterates h-outer so the PE consumes w2 tiles as they arrive instead
    of waiting for the whole expert.
  - Bias+relu / bias+copy epilogues alternate between ScalarE and
    VectorE so neither engine becomes the drain bottleneck.
"""

import numpy as np
import ml_dtypes

D = 1024
H = 2048
E = 16
NCORES = 8
P = 128
DB = D // P   # 8 d-chunks
HB = H // P   # 16 h-chunks

_program_cache = {}


def _build_program(C):
    """Trace the per-core Bass/Tile program for token capacity C (<=512)."""
    import concourse.tile as tile
    from concourse import bacc, mybir

    assert C <= 512
    f32 = mybir.dt.float32
    bf16 = mybir.dt.bfloat16
    AF = mybir.ActivationFunctionType
    ALU = mybir.AluOpType

    nc = bacc.Bacc(
        "TRN2", target_bir_lowering=False, debug=False, num_devices=NCORES
    )
    xT = nc.dram_tensor("xT", [D, 2 * C], bf16, kind="ExternalInput").ap()
    w1s = nc.dram_tensor("w1s", [2, D, H], bf16, kind="ExternalInput").ap()
    w2s = nc.dram_tensor("w2s", [2, H, D], bf16, kind="ExternalInput").ap()
    b1s = nc.dram_tensor("b1s", [2, P, HB], f32, kind="ExternalInput").ap()
    b2s = nc.dram_tensor("b2s", [2, P, DB], f32, kind="ExternalInput").ap()
    yT = nc.dram_tensor("yT", [2, D, C], bf16, kind="ExternalOutput").ap()

    with tile.TileContext(nc) as tc:
        with (
            tc.tile_pool(name="xp", bufs=16) as xp,
            tc.tile_pool(name="w1p", bufs=16) as w1p,
            tc.tile_pool(name="w2p", bufs=32) as w2p,
            tc.tile_pool(name="hp", bufs=32) as hp,
            tc.tile_pool(name="bp", bufs=4) as bp,
            tc.tile_pool(name="yp", bufs=8) as yp,
            tc.tile_pool(name="ps", bufs=8, space="PSUM") as ps,
        ):
            xts = [[None] * DB for _ in range(2)]
            w1ts = [[None] * DB for _ in range(2)]
            hts = [[None] * HB for _ in range(2)]
            b1ts = [None, None]
            b2ts = [None, None]

            # Input DMAs. Weights ride the sync HWDGE queue in exact
            # consumption order; small xT/bias tiles go via gpsimd.
            for e in range(2):
                b1t = bp.tile([P, HB], f32, tag="b1")
                nc.gpsimd.dma_start(b1t[:], b1s[e])
                b1ts[e] = b1t
                b2t = bp.tile([P, DB], f32, tag="b2")
                nc.gpsimd.dma_start(b2t[:], b2s[e])
                b2ts[e] = b2t
                for d in range(DB):
                    xt = xp.tile([P, C], bf16, tag="xT")
                    nc.gpsimd.dma_start(
                        xt[:], xT[d * P:(d + 1) * P, e * C:(e + 1) * C]
                    )
                    xts[e][d] = xt
                    w1t = w1p.tile([P, H], bf16, tag="w1")
                    nc.sync.dma_start(w1t[:], w1s[e, d * P:(d + 1) * P, :])
                    w1ts[e][d] = w1t

            def epilogue(i, out_t, acc_t, bias_col, relu):
                """Bias (+relu) from PSUM to SBUF, alternating engines."""
                if i % 2 == 0:
                    nc.scalar.activation(
                        out_t[:], acc_t[:],
                        AF.Relu if relu else AF.Identity,
                        bias=bias_col,
                    )
                elif relu:
                    nc.vector.tensor_scalar(
                        out_t[:], acc_t[:], bias_col, 0.0, ALU.add, ALU.max
                    )
                else:
                    nc.vector.tensor_scalar_add(out_t[:], acc_t[:], bias_col)

            for e in range(2):
                # ---- stage 1: hT = relu(w1.T @ xT + b1) ----
                for h in range(HB):
                    acc = ps.tile([P, C], f32, tag="acc")
                    for d in range(DB):
                        nc.tensor.matmul(
                            acc[:],
                            lhsT=w1ts[e][d][:, h * P:(h + 1) * P],
                            rhs=xts[e][d][:],
                            start=(d == 0),
                            stop=(d == DB - 1),
                        )
                    ht = hp.tile([P, C], bf16, tag="hT")
                    epilogue(h, ht, acc, b1ts[e][:, h:h + 1], relu=True)
                    hts[e][h] = ht

                # ---- stage 2: yT = w2.T @ hT + b2 (h-outer so the PE
                # consumes each w2 h-tile as soon as its DMA lands) ----
                accs = [
                    ps.tile([P, C], f32, tag="acc", name=f"acc2_{e}_{d}")
                    for d in range(DB)
                ]
                for h in range(HB):
                    w2t = w2p.tile([P, D], bf16, tag="w2")
                    nc.sync.dma_start(w2t[:], w2s[e, h * P:(h + 1) * P, :])
                    for d in range(DB):
                        nc.tensor.matmul(
                            accs[d][:],
                            lhsT=w2t[:, d * P:(d + 1) * P],
                            rhs=hts[e][h][:],
                            start=(h == 0),
                            stop=(h == HB - 1),
                        )
                for d in range(DB):
                    yt = yp.tile([P, C], bf16, tag="yt")
                    epilogue(d, yt, accs[d], b2ts[e][:, d:d + 1], relu=False)
                    nc.scalar.dma_start(yT[e, d * P:(d + 1) * P, :], yt[:])

    nc.compile()
    return nc


def kernel(x, gate_w, gate_b, w1, b1, w2, b2, _trace=False):
    from concourse.bass_utils import run_bass_kernel_spmd

    x = np.asarray(x, dtype=np.float32)
    B, S, d_in = x.shape
    T = B * S
    xf = x.reshape(T, d_in)

    # --- routing (host side: this is the dispatch/sharding step) ---
    logits = xf @ np.asarray(gate_w, dtype=np.float32) + np.asarray(
        gate_b, dtype=np.float32
    )
    top1 = np.argmax(logits, axis=-1)
    idxs = [np.nonzero(top1 == e)[0] for e in range(E)]
    C = max(32, max(len(i) for i in idxs))
    C = (C + 31) // 32 * 32
    C = min(C, 512)
    assert all(len(i) <= C for i in idxs), "expert capacity overflow"

    if C not in _program_cache:
        _program_cache[C] = _build_program(C)
    nc = _program_cache[C]

    bf16 = ml_dtypes.bfloat16
    w1 = np.asarray(w1)
    w2 = np.asarray(w2)
    b1 = np.asarray(b1, dtype=np.float32)
    b2 = np.asarray(b2, dtype=np.float32)

    in_maps = []
    for core in range(NCORES):
        xT = np.zeros((D, 2 * C), dtype=bf16)
        w1s = np.empty((2, D, H), dtype=bf16)
        w2s = np.empty((2, H, D), dtype=bf16)
        b1s = np.empty((2, P, HB), dtype=np.float32)
        b2s = np.empty((2, P, DB), dtype=np.float32)
        for s in range(2):
            e = 2 * core + s
            idx = idxs[e]
            if len(idx):
                xT[:, s * C:s * C + len(idx)] = xf[idx].T.astype(bf16)
            w1s[s] = w1[e].astype(bf16)
            w2s[s] = w2[e].astype(bf16)
            b1s[s] = b1[e].reshape(HB, P).T
            b2s[s] = b2[e].reshape(DB, P).T
        in_maps.append(
            {"xT": xT, "w1s": w1s, "w2s": w2s, "b1s": b1s, "b2s": b2s}
        )

    res = run_bass_kernel_spmd(
        nc, in_maps, core_ids=list(range(NCORES)), trace=_trace
    )

    out = np.zeros((T, D), dtype=np.float32)
    for core in range(NCORES):
        yT_out = res.results[core]["yT"]
        for s in range(2):
            e = 2 * core + s
            idx = idxs[e]
            if len(idx):
                out[idx] = yT_out[s][:, :len(idx)].T.astype(np.float32)
    if _trace:
        kernel.last_result = res
    return out.reshape(B, S, D)
